# revision 4
# baseline (speedup 1.0000x reference)
"""CapsuleLayer dynamic-routing kernel for TRN2, 8 NeuronCores, batch-sharded.

Per core: B_loc=8, I=2048, K=16, D=8, E=16.
SBUF layout: partitions p = i_sub*8 + b (16 i's per block x 8 batches), 128 j-blocks.
u_hat created via block-diagonal matmuls (stationary = blkdiag(inputs), moving = W),
routing sums via blkdiag-ones matmuls with PSUM accumulation; softmax/squash on DVE/ACT.
Host pre-packs all layouts (bf16 cast + transpose + blkdiag) in numpy.
"""
import sys
sys.path.insert(0, "/opt/trn_rl_repo")

import numpy as np
import ml_dtypes

import concourse.bass as bass
import concourse.tile as tile
from concourse import bacc, mybir
from concourse.bass_utils import run_bass_kernel_spmd

NCORES = 8
B, I, K, D, E = 64, 2048, 16, 8, 16
BL = B // NCORES          # 8 batches per core
NJ = I // 16              # 128 blocks of 16 input capsules
JC = 16                   # j-blocks per routing chunk
EPS = 1e-7

bf16 = mybir.dt.bfloat16
f32 = mybir.dt.float32
FT = mybir.ActivationFunctionType

TRACE = False
_NC_CACHE = {}


def _bc(ap, shape):
    try:
        return ap.broadcast_to(shape)
    except Exception:
        return ap.to_broadcast(shape)


def _capsule_kernel(tc, vout, ablk, wmv, onesa, onesb):
    nc = tc.nc
    with (
        tc.tile_pool(name="singles", bufs=1) as singles,
        tc.tile_pool(name="wstream", bufs=6) as wpool,
        tc.tile_pool(name="crps", bufs=5, space="PSUM") as crps,
        tc.tile_pool(name="sps", bufs=2, space="PSUM") as sps,
        tc.tile_pool(name="chunk", bufs=2) as chpool,
        tc.tile_pool(name="small", bufs=3) as small,
        tc.tile_pool(name="vreps", bufs=2) as vreps,
    ):
        ones_a = singles.tile([128, 8], bf16)
        nc.sync.dma_start(out=ones_a, in_=onesa)
        ones_b = singles.tile([128, 8], bf16)
        nc.sync.dma_start(out=ones_b, in_=onesb)
        ablk_sb = singles.tile([128, NJ, 128], bf16)
        nc.sync.dma_start(out=ablk_sb, in_=ablk)

        u_bf = singles.tile([128, NJ, K, E], bf16)      # 8 MiB
        logits = singles.tile([128, NJ, K], f32)        # 1 MiB

        # ---- phase 1: u_hat creation + s0 = (1/16) sum_i u_hat ----
        s_ps = sps.tile([8, K, E], f32)
        for j in range(NJ):
            wt = wpool.tile([128, 256], bf16)
            nc.sync.dma_start(out=wt, in_=wmv[j])
            ps = crps.tile([128, K, E], f32)
            nc.tensor.matmul(ps, lhsT=ablk_sb[:, j], rhs=wt,
                             start=True, stop=True, skip_group_check=True)
            if j % 2 == 0:
                nc.vector.tensor_copy(u_bf[:, j], ps)
            else:
                nc.scalar.activation(u_bf[:, j], ps, func=FT.Copy)
            nc.tensor.matmul(s_ps, lhsT=ones_a, rhs=u_bf[:, j],
                             start=(j == 0), stop=(j == NJ - 1),
                             skip_group_check=True)

        def squash(s_psum, make_rep):
            s_sb = small.tile([8, K, E], f32, tag="s_sb")
            nc.vector.tensor_copy(s_sb, s_psum)
            sq = small.tile([8, K, E], f32, tag="sq")
            nc.vector.tensor_mul(sq, s_sb, s_sb)
            t8 = small.tile([8, K, 8], f32, tag="sq8")
            nc.vector.tensor_add(t8, sq[:, :, 0:8], sq[:, :, 8:16])
            t4 = small.tile([8, K, 4], f32, tag="sq4")
            nc.vector.tensor_add(t4, t8[:, :, 0:4], t8[:, :, 4:8])
            t2 = small.tile([8, K, 2], f32, tag="sq2")
            nc.vector.tensor_add(t2, t4[:, :, 0:2], t4[:, :, 2:4])
            sn = small.tile([8, K], f32, tag="sn")
            nc.vector.tensor_add(sn, t2[:, :, 0], t2[:, :, 1])
            sne = small.tile([8, K], f32, tag="sne")
            nc.vector.tensor_scalar_add(sne, sn, EPS)
            sqr = small.tile([8, K], f32, tag="sqr")
            nc.scalar.activation(sqr, sne, func=FT.Sqrt)
            onep = small.tile([8, K], f32, tag="onep")
            nc.vector.tensor_scalar_add(onep, sn, 1.0)
            den = small.tile([8, K], f32, tag="den")
            nc.vector.tensor_mul(den, sqr, onep)
            rec = small.tile([8, K], f32, tag="recd")
            nc.vector.reciprocal(rec, den)
            fac = small.tile([8, K], f32, tag="fac")
            nc.vector.tensor_mul(fac, sn, rec)
            v_sb = small.tile([8, K, E], f32, tag="v_sb")
            nc.vector.tensor_mul(v_sb, s_sb, _bc(fac.unsqueeze(2), [8, K, E]))
            if not make_rep:
                return v_sb, None
            v_rep = vreps.tile([128, K, E], bf16, tag="v_rep")
            nc.vector.tensor_copy(v_rep[0:8], v_sb)
            for g in range(1, 16):
                nc.sync.dma_start(out=v_rep[8 * g:8 * g + 8], in_=v_rep[0:8])
            return v_sb, v_rep

        _, v_rep = squash(s_ps, True)

        # ---- routing iterations ----
        v_final = None
        for r in (1, 2):
            s_ps = sps.tile([8, K, E], f32)
            for ci in range(NJ // JC):
                jsl = slice(ci * JC, (ci + 1) * JC)
                # agreement: logits[:, jsl, k] (+)= sum_e u*v
                prod = chpool.tile([128, JC, K, E], bf16, tag="prod")
                nc.gpsimd.tensor_mul(
                    prod, u_bf[:, jsl],
                    _bc(v_rep.unsqueeze(1), [128, JC, K, E]))
                a8 = chpool.tile([128, JC, K, 8], bf16, tag="a8")
                nc.vector.tensor_add(a8, prod[:, :, :, 0:8], prod[:, :, :, 8:16])
                a4 = chpool.tile([128, JC, K, 4], bf16, tag="a4")
                nc.vector.tensor_add(a4, a8[:, :, :, 0:4], a8[:, :, :, 4:8])
                a2 = chpool.tile([128, JC, K, 2], bf16, tag="a2")
                nc.vector.tensor_add(a2, a4[:, :, :, 0:2], a4[:, :, :, 2:4])
                if r == 1:
                    nc.vector.tensor_add(logits[:, jsl], a2[:, :, :, 0], a2[:, :, :, 1])
                else:
                    a1 = chpool.tile([128, JC, K], f32, tag="a1")
                    nc.vector.tensor_add(a1, a2[:, :, :, 0], a2[:, :, :, 1])
                    nc.vector.tensor_add(logits[:, jsl], logits[:, jsl], a1)
                # softmax over k
                ex = chpool.tile([128, JC, K], f32, tag="ex")
                nc.scalar.activation(ex, logits[:, jsl], func=FT.Exp)
                k8 = chpool.tile([128, JC, 8], f32, tag="k8")
                nc.vector.tensor_add(k8, ex[:, :, 0:8], ex[:, :, 8:16])
                k4 = chpool.tile([128, JC, 4], f32, tag="k4")
                nc.vector.tensor_add(k4, k8[:, :, 0:4], k8[:, :, 4:8])
                k2 = chpool.tile([128, JC, 2], f32, tag="k2")
                nc.vector.tensor_add(k2, k4[:, :, 0:2], k4[:, :, 2:4])
                ks = chpool.tile([128, JC], f32, tag="ks")
                nc.vector.tensor_add(ks, k2[:, :, 0], k2[:, :, 1])
                krec = chpool.tile([128, JC], f32, tag="krec")
                nc.vector.reciprocal(krec, ks)
                cch = chpool.tile([128, JC, K], bf16, tag="cch")
                nc.vector.tensor_mul(cch, ex, _bc(krec.unsqueeze(2), [128, JC, K]))
                cu = chpool.tile([128, JC, K, E], bf16, tag="cu")
                nc.vector.tensor_mul(cu, u_bf[:, jsl],
                                     _bc(cch.unsqueeze(3), [128, JC, K, E]))
                for jj in range(JC):
                    nc.tensor.matmul(
                        s_ps, lhsT=ones_b, rhs=cu[:, jj],
                        start=(ci == 0 and jj == 0),
                        stop=(ci == NJ // JC - 1 and jj == JC - 1),
                        skip_group_check=True)
            v_sb, v_rep = squash(s_ps, r != 2)
            v_final = v_sb

        nc.sync.dma_start(out=vout, in_=v_final)


def _build():
    if "nc" in _NC_CACHE:
        return _NC_CACHE["nc"]
    nc = bacc.Bacc("TRN2", target_bir_lowering=False, debug=False,
                   num_devices=NCORES)
    ablk = nc.dram_tensor("ablk", [128, NJ, 128], bf16, kind="ExternalInput").ap()
    wmv = nc.dram_tensor("wmv", [NJ, 128, 256], bf16, kind="ExternalInput").ap()
    onesa = nc.dram_tensor("onesa", [128, 8], bf16, kind="ExternalInput").ap()
    onesb = nc.dram_tensor("onesb", [128, 8], bf16, kind="ExternalInput").ap()
    vout = nc.dram_tensor("vout", [BL, K, E], f32, kind="ExternalOutput").ap()
    with tile.TileContext(nc) as tc:
        _capsule_kernel(tc, vout, ablk, wmv, onesa, onesb)
    nc.compile()
    _NC_CACHE["nc"] = nc
    return nc


def kernel(inputs, W):
    inputs = np.asarray(inputs, np.float32)
    W = np.asarray(W, np.float32)
    nc = _build()

    # W[i,k,d,e] -> [j, (i16 d), (k e)] bf16, contiguous per block
    Wb = np.ascontiguousarray(
        W.reshape(NJ, 16, K, D, E).transpose(0, 1, 3, 2, 4)
    ).reshape(NJ, 128, 256).astype(ml_dtypes.bfloat16)

    onesa_np = np.zeros((128, 8), np.float32)
    onesa_np[np.arange(128), np.arange(128) % 8] = 1.0 / 16.0
    onesb_np = (onesa_np * 16.0).astype(ml_dtypes.bfloat16)
    onesa_np = onesa_np.astype(ml_dtypes.bfloat16)

    in_maps = []
    for c in range(NCORES):
        inp_c = inputs[c * BL:(c + 1) * BL]          # [8, 2048, 8]
        inp_t = inp_c.reshape(BL, NJ, 16, D)          # b, j, iu, d
        ab = np.zeros((16, D, NJ, 16, BL), np.float32)  # iu d j iu2 b
        for iu in range(16):
            ab[iu, :, :, iu, :] = inp_t[:, :, iu, :].transpose(2, 1, 0)
        ab = ab.reshape(128, NJ, 128).astype(ml_dtypes.bfloat16)
        in_maps.append({"ablk": ab, "wmv": Wb,
                        "onesa": onesa_np, "onesb": onesb_np})

    br = run_bass_kernel_spmd(nc, in_maps, core_ids=list(range(NCORES)),
                              trace=TRACE)
    if br.exec_time_ns is not None:
        print(f"HW exec time: {br.exec_time_ns} ns")
    out = np.concatenate([r["vout"] for r in br.results], axis=0)
    return out.astype(np.float32)


# revision 7
# speedup vs baseline: 1.0653x; 1.0653x over previous
"""CapsuleLayer dynamic-routing kernel for TRN2, 8 NeuronCores, batch-sharded.

Per core: B_loc=8, I=2048, K=16, D=8, E=16.
SBUF layout: partitions p = i_sub*8 + b (16 i's per block x 8 batches), 128 j-blocks.
u_hat created via block-diagonal matmuls (stationary = blkdiag(inputs), moving = W),
routing sums via blkdiag-ones matmuls with PSUM accumulation; softmax/squash on DVE/ACT.
Host pre-packs all layouts (bf16 cast + transpose + blkdiag) in numpy.
"""
import sys
sys.path.insert(0, "/opt/trn_rl_repo")

import numpy as np
import ml_dtypes

import concourse.bass as bass
import concourse.tile as tile
from concourse import bacc, mybir
from concourse.bass_utils import run_bass_kernel_spmd

NCORES = 8
B, I, K, D, E = 64, 2048, 16, 8, 16
BL = B // NCORES          # 8 batches per core
NJ = I // 16              # 128 blocks of 16 input capsules
JC = 16                   # j-blocks per routing chunk
EPS = 1e-7

bf16 = mybir.dt.bfloat16
f32 = mybir.dt.float32
FT = mybir.ActivationFunctionType

TRACE = False
_NC_CACHE = {}


def _bc(ap, shape):
    try:
        return ap.broadcast_to(shape)
    except Exception:
        return ap.to_broadcast(shape)


def _capsule_kernel(tc, vout, ablk, wmv, onesa, onesb):
    nc = tc.nc
    with (
        tc.tile_pool(name="singles", bufs=1) as singles,
        tc.tile_pool(name="wstream", bufs=6) as wpool,
        tc.tile_pool(name="crps", bufs=5, space="PSUM") as crps,
        tc.tile_pool(name="sps", bufs=2, space="PSUM") as sps,
        tc.tile_pool(name="chunk", bufs=3) as chpool,
        tc.tile_pool(name="small", bufs=3) as small,
        tc.tile_pool(name="vreps", bufs=2) as vreps,
    ):
        ones_a = singles.tile([128, 8], bf16)
        nc.sync.dma_start(out=ones_a, in_=onesa)
        ones_b = singles.tile([128, 8], bf16)
        nc.sync.dma_start(out=ones_b, in_=onesb)
        ablk_sb = singles.tile([128, NJ, 128], bf16)
        nc.sync.dma_start(out=ablk_sb, in_=ablk)

        u_bf = singles.tile([128, NJ, K, E], bf16)      # 8 MiB
        logits = singles.tile([128, NJ, K], f32)        # 1 MiB

        # ---- phase 1: u_hat creation + s0 = (1/16) sum_i u_hat ----
        s_ps = sps.tile([8, K, E], f32)
        for j in range(NJ):
            wt = wpool.tile([128, 256], bf16)
            nc.sync.dma_start(out=wt, in_=wmv[j])
            ps = crps.tile([128, K, E], f32)
            nc.tensor.matmul(ps, lhsT=ablk_sb[:, j], rhs=wt,
                             start=True, stop=True, skip_group_check=True)
            if j % 2 == 0:
                nc.vector.tensor_copy(u_bf[:, j], ps)
            else:
                nc.scalar.activation(u_bf[:, j], ps, func=FT.Copy)
            nc.tensor.matmul(s_ps, lhsT=ones_a, rhs=u_bf[:, j],
                             start=(j == 0), stop=(j == NJ - 1),
                             skip_group_check=True)

        def squash(s_psum, make_rep):
            s_sb = small.tile([8, K, E], f32, tag="s_sb")
            nc.vector.tensor_copy(s_sb, s_psum)
            sq = small.tile([8, K, E], f32, tag="sq")
            nc.vector.tensor_mul(sq, s_sb, s_sb)
            t8 = small.tile([8, K, 8], f32, tag="sq8")
            nc.vector.tensor_add(t8, sq[:, :, 0:8], sq[:, :, 8:16])
            t4 = small.tile([8, K, 4], f32, tag="sq4")
            nc.vector.tensor_add(t4, t8[:, :, 0:4], t8[:, :, 4:8])
            t2 = small.tile([8, K, 2], f32, tag="sq2")
            nc.vector.tensor_add(t2, t4[:, :, 0:2], t4[:, :, 2:4])
            sn = small.tile([8, K], f32, tag="sn")
            nc.vector.tensor_add(sn, t2[:, :, 0], t2[:, :, 1])
            sne = small.tile([8, K], f32, tag="sne")
            nc.vector.tensor_scalar_add(sne, sn, EPS)
            sqr = small.tile([8, K], f32, tag="sqr")
            nc.scalar.activation(sqr, sne, func=FT.Sqrt)
            onep = small.tile([8, K], f32, tag="onep")
            nc.vector.tensor_scalar_add(onep, sn, 1.0)
            den = small.tile([8, K], f32, tag="den")
            nc.vector.tensor_mul(den, sqr, onep)
            rec = small.tile([8, K], f32, tag="recd")
            nc.vector.reciprocal(rec, den)
            fac = small.tile([8, K], f32, tag="fac")
            nc.vector.tensor_mul(fac, sn, rec)
            v_sb = small.tile([8, K, E], f32, tag="v_sb")
            nc.vector.tensor_mul(v_sb, s_sb, _bc(fac.unsqueeze(2), [8, K, E]))
            if not make_rep:
                return v_sb, None
            v_rep = vreps.tile([128, K, E], bf16, tag="v_rep")
            nc.vector.tensor_copy(v_rep[0:8], v_sb)
            for g in range(1, 16):
                nc.sync.dma_start(out=v_rep[8 * g:8 * g + 8], in_=v_rep[0:8])
            return v_sb, v_rep

        _, v_rep = squash(s_ps, True)

        # ---- routing iterations ----
        v_final = None
        for r in (1, 2):
            s_ps = sps.tile([8, K, E], f32)
            for ci in range(NJ // JC):
                jsl = slice(ci * JC, (ci + 1) * JC)
                # agreement: logits[:, jsl, k] (+)= sum_e u*v
                prod = chpool.tile([128, JC, K, E], bf16, tag="prod")
                peng = nc.gpsimd if ci % 2 == 0 else nc.vector
                peng.tensor_mul(
                    prod, u_bf[:, jsl],
                    _bc(v_rep.unsqueeze(1), [128, JC, K, E]))
                a8 = chpool.tile([128, JC, K, 8], bf16, tag="a8")
                nc.vector.tensor_add(a8, prod[:, :, :, 0:8], prod[:, :, :, 8:16])
                a4 = chpool.tile([128, JC, K, 4], bf16, tag="a4")
                nc.vector.tensor_add(a4, a8[:, :, :, 0:4], a8[:, :, :, 4:8])
                a2 = chpool.tile([128, JC, K, 2], bf16, tag="a2")
                nc.vector.tensor_add(a2, a4[:, :, :, 0:2], a4[:, :, :, 2:4])
                if r == 1:
                    nc.vector.tensor_add(logits[:, jsl], a2[:, :, :, 0], a2[:, :, :, 1])
                else:
                    a1 = chpool.tile([128, JC, K], f32, tag="a1")
                    nc.vector.tensor_add(a1, a2[:, :, :, 0], a2[:, :, :, 1])
                    nc.vector.tensor_add(logits[:, jsl], logits[:, jsl], a1)
                # softmax over k
                ex = chpool.tile([128, JC, K], f32, tag="ex")
                nc.scalar.activation(ex, logits[:, jsl], func=FT.Exp)
                k8 = chpool.tile([128, JC, 8], f32, tag="k8")
                nc.vector.tensor_add(k8, ex[:, :, 0:8], ex[:, :, 8:16])
                k4 = chpool.tile([128, JC, 4], f32, tag="k4")
                nc.vector.tensor_add(k4, k8[:, :, 0:4], k8[:, :, 4:8])
                k2 = chpool.tile([128, JC, 2], f32, tag="k2")
                nc.vector.tensor_add(k2, k4[:, :, 0:2], k4[:, :, 2:4])
                ks = chpool.tile([128, JC], f32, tag="ks")
                nc.vector.tensor_add(ks, k2[:, :, 0], k2[:, :, 1])
                krec = chpool.tile([128, JC], f32, tag="krec")
                nc.vector.reciprocal(krec, ks)
                cch = chpool.tile([128, JC, K], bf16, tag="cch")
                nc.vector.tensor_mul(cch, ex, _bc(krec.unsqueeze(2), [128, JC, K]))
                cu = chpool.tile([128, JC, K, E], bf16, tag="cu")
                cueng = nc.vector if ci % 2 == 0 else nc.gpsimd
                cueng.tensor_mul(cu, u_bf[:, jsl],
                                 _bc(cch.unsqueeze(3), [128, JC, K, E]))
                for jj in range(JC):
                    nc.tensor.matmul(
                        s_ps, lhsT=ones_b, rhs=cu[:, jj],
                        start=(ci == 0 and jj == 0),
                        stop=(ci == NJ // JC - 1 and jj == JC - 1),
                        skip_group_check=True)
            v_sb, v_rep = squash(s_ps, r != 2)
            v_final = v_sb

        nc.sync.dma_start(out=vout, in_=v_final)


def _build():
    if "nc" in _NC_CACHE:
        return _NC_CACHE["nc"]
    nc = bacc.Bacc("TRN2", target_bir_lowering=False, debug=False,
                   num_devices=NCORES)
    ablk = nc.dram_tensor("ablk", [128, NJ, 128], bf16, kind="ExternalInput").ap()
    wmv = nc.dram_tensor("wmv", [NJ, 128, 256], bf16, kind="ExternalInput").ap()
    onesa = nc.dram_tensor("onesa", [128, 8], bf16, kind="ExternalInput").ap()
    onesb = nc.dram_tensor("onesb", [128, 8], bf16, kind="ExternalInput").ap()
    vout = nc.dram_tensor("vout", [BL, K, E], f32, kind="ExternalOutput").ap()
    with tile.TileContext(nc) as tc:
        _capsule_kernel(tc, vout, ablk, wmv, onesa, onesb)
    nc.compile()
    _NC_CACHE["nc"] = nc
    return nc


def kernel(inputs, W):
    inputs = np.asarray(inputs, np.float32)
    W = np.asarray(W, np.float32)
    nc = _build()

    # W[i,k,d,e] -> [j, (i16 d), (k e)] bf16, contiguous per block
    Wb = np.ascontiguousarray(
        W.reshape(NJ, 16, K, D, E).transpose(0, 1, 3, 2, 4)
    ).reshape(NJ, 128, 256).astype(ml_dtypes.bfloat16)

    onesa_np = np.zeros((128, 8), np.float32)
    onesa_np[np.arange(128), np.arange(128) % 8] = 1.0 / 16.0
    onesb_np = (onesa_np * 16.0).astype(ml_dtypes.bfloat16)
    onesa_np = onesa_np.astype(ml_dtypes.bfloat16)

    in_maps = []
    for c in range(NCORES):
        inp_c = inputs[c * BL:(c + 1) * BL]          # [8, 2048, 8]
        inp_t = inp_c.reshape(BL, NJ, 16, D)          # b, j, iu, d
        ab = np.zeros((16, D, NJ, 16, BL), np.float32)  # iu d j iu2 b
        for iu in range(16):
            ab[iu, :, :, iu, :] = inp_t[:, :, iu, :].transpose(2, 1, 0)
        ab = ab.reshape(128, NJ, 128).astype(ml_dtypes.bfloat16)
        in_maps.append({"ablk": ab, "wmv": Wb,
                        "onesa": onesa_np, "onesb": onesb_np})

    br = run_bass_kernel_spmd(nc, in_maps, core_ids=list(range(NCORES)),
                              trace=TRACE)
    if br.exec_time_ns is not None:
        print(f"HW exec time: {br.exec_time_ns} ns")
    out = np.concatenate([r["vout"] for r in br.results], axis=0)
    return out.astype(np.float32)
